# revision 45
# baseline (speedup 1.0000x reference)
"""Trainium2 Bass kernel for nn_MemoryReader (memory-reader cross attention).

Reference computation (per batch b):
    ab       = mk[b].T @ qk[b] / sqrt(CK)      # [N_mem, N_q], N_mem=9216, N_q=2304
    affinity = softmax(ab, axis=0)             # over memory axis
    val      = mv[b] @ affinity                # [CV, N_q]
Output reshaped to [B, TQ, CV, H, W] = [4, 1, 512, 48, 48].

Sharding: 8 cores = 4 batches x 2 query-halves (softmax is over the memory
axis, so query columns are independent). Each core computes a [512, 1152]
output shard.

Per-core kernel (all big matmuls bf16, f32 accumulation):
  0. Host pre-casts mk/qk/mv to bf16 (mk/qk zero-padded to 128 partitions).
  1. mvT[p,i,c] = mv[c, i*128+p] built with 72 XBAR DMA-transposes straight
     from HBM (2-byte dtype requirement is why mv ships as bf16).
  2. For each query chunk (3 x 384), loop the 72 memory tiles:
       mm1:  ab_psum[128,384] = mk_blk.T @ qk_chunk
       exp:  ScalarE Exp with scale=1/8 -> expab[:, i, :] bf16
             (no max subtraction: logits ~N(0,1), max ~4.5 -> exp safe)
       mm2:  4 accumulating matmuls out_psum[cv] += mvT_blk.T @ expab_i
     colsum: one strided DVE reduce over the tile axis, then a ones-matmul
     partition-reduce, reciprocal, E0-matmul broadcast, and 4 multiplies.
  3. DMA out_sbuf [128, 4, 384] -> out [512, 1152] per chunk.

NOTE on sync: walrus allows only ONE semaphore wait on most engine/DMA
instructions (PE Matmult gets ~2 more via the LDWEIGHTS/MATMUL split). The
kernel is structured so no instruction needs more than its limit: DMAs only
ever write fresh buffers (zero waits), per-chunk work tiles use bufs=3 (one
per chunk, never reused), and a small ScalarE absorber op at each chunk end
makes ACT observe the DVE reduce before the next chunk's first exp.
"""

import numpy as np

B, CK, CV, T, H, W = 4, 64, 512, 4, 48, 48
TQ = 1
NMEM = T * H * W          # 9216
NQH = 1152                # queries per core (half of 2304)
P = 128
NT = NMEM // P            # 72 memory tiles
MCH = 384                 # query chunk size
NCH = NQH // MCH          # 3 chunks
NCV = CV // P             # 4 cv tiles

_CACHED = {}


def _build_bass():
    import concourse.bass as bass
    import concourse.mybir as mybir
    import concourse.tile as tile

    f32 = mybir.dt.float32
    bf16 = mybir.dt.bfloat16

    nc = bass.Bass()
    # inputs pre-cast to bf16 on host; mk/qk merged into one tensor (keeps
    # the DMA count at 3 so the tail drain stays under its 8-wait limit),
    # zero-padded to 128 partitions; mv pre-transposed on host to [NMEM, CV]
    mkqk_d = nc.dram_tensor("mkqk", [P, NMEM + NQH], bf16, kind="ExternalInput")
    mvt_d = nc.dram_tensor("mvt", [NMEM, CV], bf16, kind="ExternalInput")
    out_d = nc.dram_tensor("out", [CV, NQH], f32, kind="ExternalOutput")

    with tile.TileContext(nc) as tc:
        with (
            tc.tile_pool(name="const", bufs=1) as const,
            tc.tile_pool(name="big", bufs=1) as big,
            tc.tile_pool(name="work", bufs=3) as work,
            tc.tile_pool(name="cs", bufs=2) as cs_pool,
            tc.tile_pool(name="ps_ab", bufs=4, space="PSUM") as ps_ab,
            tc.tile_pool(name="ps_out", bufs=1, space="PSUM") as ps_out,
        ):
            ones_col = const.tile([P, 1], f32, name="ones_col")
            nc.vector.memset(ones_col, 1.0)
            e0 = const.tile([P, P], f32, name="e0")
            nc.vector.memset(e0, 0.0)
            nc.vector.memset(e0[:1, :], 1.0)
            # persistent reciprocal row; rows 1..127 stay zero forever so the
            # E0-matmul broadcast never multiplies garbage. Zeroed on ScalarE
            # so the per-chunk DVE reciprocal never needs a DVE self-wait.
            recip = const.tile([P, MCH], f32, name="recip")
            nc.scalar.memzero(recip)

            # --- inputs (qk first so the first slice unblocks mm1 quickly) ---
            mkqk_b = big.tile([P, NMEM + NQH], bf16, name="mkqk_b")
            CSL = (NMEM + NQH) // 8
            for k in range(8):
                nc.sync.dma_start(
                    mkqk_b[:, k * CSL : (k + 1) * CSL],
                    mkqk_d[:, k * CSL : (k + 1) * CSL],
                )
            qk_b = mkqk_b[:, :NQH]
            mk_b = mkqk_b[:, NQH:]

            # split into 8 slices so the first mm2s can start as soon as the
            # first slice lands instead of waiting for the whole 9.4 MB
            mvT = big.tile([P, NT, CV], bf16, name="mvT")
            NSL = NT // 9  # 8 slices of 9 tiles
            for k in range(NSL):
                nc.sync.dma_start(
                    mvT[:, k * 9 : (k + 1) * 9, :],
                    mvt_d[k * 9 * P : (k + 1) * 9 * P, :].rearrange(
                        "(n p) c -> p n c", p=P
                    ),
                )

            # single persistent output accumulator; one store at the end
            out_sb = big.tile([P, NCV, NQH], f32, name="out_sb")

            # --- main fused loop over query chunks ---
            prev_csum = None
            for mc in range(NCH):
                qs = slice(mc * MCH, (mc + 1) * MCH)
                expab = big.tile([P, NT, MCH], bf16, name="expab", tag="expab")
                outp = [
                    ps_out.tile([P, MCH], f32, tag=f"outp{t}", name=f"outp{t}")
                    for t in range(NCV)
                ]
                if prev_csum is not None:
                    # ACT absorber 1: observe the DVE tick (previous chunk's
                    # reduce) so exp(0) doesn't need a DVE wait for the expab
                    # slot release
                    scr_a = work.tile([1, 8], f32, tag="scr_a")
                    nc.scalar.copy(scr_a, prev_csum[:1, :8])
                # colsum runs split into pieces emitted mid-loop so the
                # DVE reduce overlaps compute instead of serializing the
                # chunk boundary; the PE-side partition-sum for piece k is
                # emitted one piece late so PE never stalls on the reduce.
                # Final pieces are small (3 tiles) to shorten the tail chain.
                bounds = [8, 17, 26, 35, 44, 53, 62, 65, 68, 71]
                csums = []
                sum_ps = None
                def mm2s(i):
                    for t in range(NCV):
                        nc.tensor.matmul(
                            outp[t],
                            lhsT=mvT[:, i, t * P : (t + 1) * P],
                            rhs=expab[:, i, :],
                            start=(i == 0),
                            stop=(i == NT - 1),
                        )

                # software-pipelined by one pair: PE stream per pair is
                # [mm1(i), mm1(i+1), mm2s(i-2), mm2s(i-1)] so the exps hide
                # behind the next pair's mm1s. The two mm1s contract K=64 on
                # disjoint row-groups (mk/qk are duplicated into partitions
                # 64..127 by the host) so they run concurrently on the PE.
                for ip in range(0, NT, 2):
                    abs_pair = []
                    for half in range(2):
                        i = ip + half
                        lo, hi = half * CK, (half + 1) * CK
                        ab = ps_ab.tile([P, MCH], f32, tag="ab")
                        nc.tensor.matmul(
                            ab,
                            lhsT=mk_b[lo:hi, i * P : (i + 1) * P],
                            rhs=qk_b[lo:hi, qs],
                            start=True,
                            stop=True,
                        )
                        abs_pair.append(ab)
                    if ip == 0 and mc > 0:
                        # ACT absorber 2: observe the PE tick (this chunk's
                        # first mm1) so exp(0) only needs its ACT self-wait
                        scr_b = work.tile([1, 8], f32, tag="scr_b")
                        nc.scalar.copy(scr_b, abs_pair[0][:1, :8])
                    for half in range(2):
                        nc.scalar.activation(
                            expab[:, ip + half, :],
                            abs_pair[half],
                            mybir.ActivationFunctionType.Exp,
                            scale=0.125,
                        )
                    if ip > 0:
                        mm2s(ip - 2)
                        mm2s(ip - 1)
                    for i in (ip, ip + 1):
                        if i in bounds:
                            k = bounds.index(i)
                            lo_t = bounds[k - 1] + 1 if k > 0 else 0
                            csum = cs_pool.tile([P, MCH], f32, tag=f"csum{k}")
                            nc.vector.reduce_sum(
                                csum[:, :, None],
                                expab[:, lo_t : i + 1, :].rearrange(
                                    "p n m -> p m n"
                                ),
                                axis=mybir.AxisListType.X,
                            )
                            csums.append(csum)
                            if sum_ps is None:
                                sum_ps = ps_ab.tile(
                                    [P, MCH], f32, tag="ab", name="sum_ps"
                                )
                            if k >= 1:
                                nc.tensor.matmul(
                                    sum_ps[:1],
                                    lhsT=ones_col,
                                    rhs=csums[k - 1],
                                    start=(k == 1),
                                    stop=False,
                                )
                mm2s(NT - 2)
                mm2s(NT - 1)
                nc.tensor.matmul(
                    sum_ps[:1],
                    lhsT=ones_col,
                    rhs=csums[-1],
                    start=False,
                    stop=True,
                )
                nc.vector.reciprocal(recip[:1], sum_ps[:1])
                bcast_ps = ps_ab.tile([P, MCH], f32, tag="ab")
                nc.tensor.matmul(
                    bcast_ps, lhsT=e0, rhs=recip, start=True, stop=True
                )
                bcast = work.tile([P, MCH], f32, tag="bcast")
                nc.vector.tensor_copy(out=bcast, in_=bcast_ps)

                for t in range(NCV):
                    nc.vector.tensor_mul(
                        out=out_sb[:, t, qs], in0=outp[t], in1=bcast
                    )
                prev_csum = csums[-1]
                # per-chunk store overlaps the next chunk's compute
                nc.sync.dma_start(
                    out_d[:, :].rearrange("(t p) m -> p t m", p=P)[:, :, qs],
                    out_sb[:, :, qs],
                )

    return nc


def _patch_sync_waits(bj):
    """Split multi-wait instructions into single-wait Drains + the original.

    The walrus build in this environment accepts at most ONE sync wait per
    instruction (two for non-transpose Matmult via the LDWEIGHTS/MATMUL
    split); Tile emits more. Semaphores are monotonic within an epoch, so
    hoisting excess waits onto same-engine Drain instructions placed
    immediately before is conservative and sound.
    """
    n = 0
    for f in bj["functions"]:
        for blk in f["blocks"]:
            out = []
            for inst in blk.get("instructions", []):
                si = inst.get("sync_info")
                w = (si or {}).get("on_wait") or []
                op = inst["opcode"]
                lim = 1
                if si and len(w) > lim:
                    extra, keep = w[:-lim], w[-lim:]
                    for k, x in enumerate(extra):
                        out.append(
                            {
                                "debug": inst.get("debug", 0),
                                "engine": inst["engine"],
                                "ins": [],
                                "outs": [],
                                "name": f"{inst['name']}-sw{k}",
                                "opcode": "Drain",
                                "sync_info": {"on_update": [], "on_wait": [x]},
                            }
                        )
                        n += 1
                    si["on_wait"] = keep
                out.append(inst)
            blk["instructions"] = out
    return n


def _get_nc():
    if "nc" not in _CACHED:
        import json

        nc = _build_bass()
        bj = json.loads(nc.to_json_bytes())
        _patch_sync_waits(bj)
        blob = json.dumps(bj).encode()
        nc.to_json_bytes = lambda: blob
        _CACHED["nc"] = nc
    return _CACHED["nc"]


def kernel(mk, qk, mv, _trace=False):
    from concourse.bass_utils import run_bass_kernel_spmd
    import ml_dtypes

    bf = ml_dtypes.bfloat16
    mkf = np.asarray(mk, dtype=np.float32).reshape(B, CK, NMEM)
    qkf = np.asarray(qk, dtype=np.float32).reshape(B, CK, TQ * H * W)
    mvf = np.asarray(mv, dtype=np.float32).reshape(B, CV, NMEM)
    # pre-cast to bf16 on host; zero-pad mk/qk partitions to 128
    NQF = TQ * H * W  # 2304
    mkqk = np.zeros((B, 2, P, NMEM + NQH), dtype=bf)  # [b, half, P, cols]
    mkbf = mkf.astype(bf)
    qkbf = qkf.astype(bf)
    for h in range(2):
        # duplicated into both partition halves for row-packed K=64 matmuls
        mkqk[:, h, :CK, :NQH] = qkbf[:, :, h * NQH : (h + 1) * NQH]
        mkqk[:, h, CK:, :NQH] = qkbf[:, :, h * NQH : (h + 1) * NQH]
        mkqk[:, h, :CK, NQH:] = mkbf
        mkqk[:, h, CK:, NQH:] = mkbf
    # host-side transpose of mv to [NMEM, CV] per batch
    mvtp = np.ascontiguousarray(mvf.transpose(0, 2, 1)).astype(bf)

    in_maps = []
    for c in range(8):
        b, h = c // 2, c % 2
        in_maps.append(
            {
                "mkqk": np.ascontiguousarray(mkqk[b, h]),
                "mvt": mvtp[b],
            }
        )

    nc = _get_nc()
    res = run_bass_kernel_spmd(nc, in_maps, core_ids=list(range(8)), trace=_trace)
    outs = [r["out"] for r in res.results]
    val = np.stack(
        [np.concatenate([outs[2 * b], outs[2 * b + 1]], axis=1) for b in range(B)]
    )  # [B, CV, 2304]
    out = val.reshape(B, CV, TQ, H, W).transpose(0, 2, 1, 3, 4)
    if _trace:
        return np.ascontiguousarray(out.astype(np.float32)), res
    return np.ascontiguousarray(out.astype(np.float32))
